# revision 9
# baseline (speedup 1.0000x reference)
"""Trainium2 Bass kernel for the GatedODEFlow problem.

Math: the reference iterates  a <- a + h*alpha(a) * (tgt - a)  where
alpha depends on a only through the low-rank projection (a - mu) @ U / S.
Since each step is a per-row convex blend toward the fixed vector tgt,
a_t = c_t * x + (1 - c_t) * tgt  for a per-row scalar c_t, and the
projection evolves affinely in c_t:

    proj_t = c_t * (x@W - tgt@W) + (tgt@W - mu@W)   with W = U / (S+1e-6)
    dist2_t = A * c_t^2 + B2 * c_t + C              (per-row A, B2; global C)
    alpha_t = exp(-dist2_t / (2*k*sigma^2))
    c_{t+1} = c_t * (1 - h * alpha_t),  c_0 = 1
    out = c_N * x + (1 - c_N) * tgt

So the device only needs ONE matmul q0 = x @ W per row plus a scalar
recurrence and a final fused blend.  The 2e-2 rel-err budget dwarfs bf16
rounding, so x is cast to bf16 right after the fp32 load (the fp32 tile
is recycled after one macroblock) and the output is stored in bf16:
HBM traffic is 64 MiB read + 32 MiB write per core -- a ~270us roofline
at 358 GB/s per core.

v5 engine layout (per 512-row macroblock):
- SP(HWDGE): the 4 x-subblock fp32 loads (2 MiB each; HWDGE sustains
  ~360 GB/s where the SWDGE fp32->bf16 cast-load path caps at ~230).
- ACT: xb = bf16(x) casts for the next macroblock, most PSUM->SBUF
  copies of transposed groups, gate Square/Identity, exp, store issues.
- PE: 128 transposes of bf16 x, 32 projection matmuls, 4 A/B matmuls.
- DVE: per subblock ttmp = (1-c)*tgt and xb *= c (both 4x-mode bf16
  tensor_scalar with a per-partition scalar AP), then one full-row
  in-place 2x-mode add xb += ttmp (tensor_tensor DOES reach 2x_1P on
  all-bf16 step-1 operands; scalar_tensor_tensor never packs and runs
  ~2.4x slower -- measured); a few copies; the scalar recurrence.
- GPSIMD: idle (its tensor ops contend for the SBUF port shared with
  the Vector engine and halve DVE 2-src throughput -- measured).

Emission is software-pipelined with consumption before production on
every engine queue (engines execute their queues strictly in order):
iteration m emits casts of m+1, blend/store of m-1, loads of m+2, then
the PE-heavy front and gate recurrence of m.

Sharding: data-parallel across 8 cores along the batch dim; small
parameters replicated (per the problem's sharding hint).
"""

import math
import os
from contextlib import ExitStack

import numpy as np
import ml_dtypes

import concourse.bass as bass
import concourse.mybir as mybir
import concourse.tile as tile
from concourse import bacc
from concourse.masks import make_identity
from concourse.bass_utils import run_bass_kernel_spmd

F32 = mybir.dt.float32
BF16 = mybir.dt.bfloat16
AF = mybir.ActivationFunctionType
OP = mybir.AluOpType

N_CORES = 8
D = 4096
KSUB = 64
SUB = 128            # rows per subblock (one partition tile)
SPM = 4              # subblocks per macroblock
MACRO = SUB * SPM    # 512 rows
DCH = 128            # d-chunk width for PE transposes
NDCH = D // DCH      # 32

DVE_COPY_OF16 = 6    # of each macro's 16 transpose groups, this many copied by DVE

_PROGRAM_CACHE: dict = {}
LAST_RESULT = None


def _build_program(rows: int, num_steps: int, neg_inv: float, exp_bias: float,
                   neg_h: float):
    nmacro = rows // MACRO
    assert rows == nmacro * MACRO, f"rows {rows} not a multiple of {MACRO}"
    assert nmacro >= 3

    nc = bacc.Bacc("TRN2")
    x_d = nc.dram_tensor("x", [rows, D], F32, kind="ExternalInput")
    w_d = nc.dram_tensor("w", [D, KSUB], BF16, kind="ExternalInput")
    tgr_d = nc.dram_tensor("tgr", [128, D], BF16, kind="ExternalInput")
    nqt_d = nc.dram_tensor("nqt", [KSUB, 1], F32, kind="ExternalInput")
    abr_d = nc.dram_tensor("abr", [128, 2], BF16, kind="ExternalInput")
    out_d = nc.dram_tensor("out", [rows, D], BF16, kind="ExternalOutput")

    with ExitStack() as ctx:
        tc = ctx.enter_context(tile.TileContext(nc))
        singles = ctx.enter_context(tc.tile_pool(name="singles", bufs=1))
        xinpool = ctx.enter_context(tc.tile_pool(name="xin", bufs=5))
        xbpool = ctx.enter_context(tc.tile_pool(name="xb", bufs=10))
        xtpool = ctx.enter_context(tc.tile_pool(name="xtp", bufs=4))
        ttpool = ctx.enter_context(tc.tile_pool(name="ttp", bufs=2))
        stkpool = ctx.enter_context(tc.tile_pool(name="stkp", bufs=2))
        smpool = ctx.enter_context(tc.tile_pool(name="smp", bufs=2))
        ptr = ctx.enter_context(tc.tile_pool(name="ptr", bufs=3, space="PSUM"))
        pq = ctx.enter_context(tc.tile_pool(name="pq", bufs=2, space="PSUM"))
        pab = ctx.enter_context(tc.tile_pool(name="pab", bufs=2, space="PSUM"))

        identu = singles.tile([128, 128], BF16)
        make_identity(nc, identu)
        w_sb = singles.tile([128, NDCH, KSUB], BF16)
        nc.sync.dma_start(out=w_sb, in_=w_d[:, :].rearrange("(j p) k -> p j k", p=128))
        tgr_sb = singles.tile([128, D], BF16)
        nc.sync.dma_start(out=tgr_sb, in_=tgr_d[:, :])
        nqt_sb = singles.tile([KSUB, 1], F32)
        nc.sync.dma_start(out=nqt_sb, in_=nqt_d[:, :])
        abr_sb = singles.tile([128, 2], BF16)
        nc.sync.dma_start(out=abr_sb, in_=abr_d[:, :])
        ebias_sb = singles.tile([128, 1], F32)
        nc.vector.memset(ebias_sb, exp_bias)

        def emit_loads(m):
            """HWDGE fp32 loads for macro m (SP queue)."""
            r0 = m * MACRO
            xins = []
            for s in range(SPM):
                xin = xinpool.tile([SUB, D], F32, tag="xin")
                nc.sync.dma_start(
                    out=xin, in_=x_d[r0 + s * SUB : r0 + (s + 1) * SUB, :])
                xins.append(xin)
            return xins

        def emit_casts(xins):
            """ACT fp32 -> bf16 casts; frees the fp32 tiles."""
            xbs = []
            for s in range(SPM):
                xb = xbpool.tile([SUB, D], BF16, tag="xb")
                nc.scalar.copy(xb, xins[s])
                xbs.append(xb)
            return xbs

        def emit_front(m, xbs):
            """PE transposes + bf16 projection + extraction + A/B."""
            q0T = pq.tile([KSUB, MACRO], F32, tag="q0T")
            for g in range(NDCH // 2):
                tp = ptr.tile([128, 2 * MACRO], BF16, tag="tp")
                for jj in range(2):
                    j = 2 * g + jj
                    for s in range(SPM):
                        nc.tensor.transpose(
                            tp[:, jj * MACRO + s * SUB
                               : jj * MACRO + (s + 1) * SUB],
                            xbs[s][:, j * DCH : (j + 1) * DCH], identu)
                xt = xtpool.tile([128, 2 * MACRO], BF16, tag="xt")
                if g < DVE_COPY_OF16:
                    nc.vector.tensor_copy(xt, tp)
                else:
                    nc.scalar.copy(xt, tp)
                nc.tensor.matmul(
                    q0T, w_sb[:, 2 * g, :], xt[:, 0:MACRO],
                    start=(g == 0), stop=False)
                nc.tensor.matmul(
                    q0T, w_sb[:, 2 * g + 1, :], xt[:, MACRO : 2 * MACRO],
                    start=False, stop=(g == NDCH // 2 - 1))

            # stk rows 0..63 = (q0T - qT)^2 ; rows 64..127 = (q0T - qT)
            stk = stkpool.tile([128, MACRO], BF16, tag="stk")
            nc.scalar.activation(stk[0:KSUB, :], q0T, AF.Square,
                                 bias=nqt_sb, scale=1.0)
            nc.scalar.activation(stk[KSUB:128, :], q0T, AF.Identity,
                                 bias=nqt_sb, scale=1.0)
            # ab[:, 2s] = A_s, ab[:, 2s+1] = B2_s
            ab = pab.tile([128, 2 * SPM], F32, tag="ab")
            for s in range(SPM):
                nc.tensor.matmul(ab[:, 2 * s : 2 * s + 2],
                                 stk[:, s * SUB : (s + 1) * SUB],
                                 abr_sb[:, 0:2], start=True, stop=True)
            return {"xbs": xbs, "ab": ab, "r0": m * MACRO}

        def emit_iteration(st):
            """Per-row scalar recurrence (DVE + ACT exp) -> c, d."""
            ab = st["ab"]
            A = ab[:, 0 : 2 * SPM : 2]
            B2 = ab[:, 1 : 2 * SPM : 2]
            c = smpool.tile([128, SPM], F32, tag="c")
            nc.vector.memset(c, 1.0)
            t1 = smpool.tile([128, SPM], F32, tag="t1")
            alpha = smpool.tile([128, SPM], F32, tag="alpha")
            for _t in range(num_steps):
                nc.vector.tensor_tensor(t1, A, c, OP.mult)
                nc.vector.tensor_tensor(t1, t1, B2, OP.add)
                nc.vector.tensor_tensor(t1, t1, c, OP.mult)
                nc.scalar.activation(alpha, t1, AF.Exp,
                                     bias=ebias_sb, scale=neg_inv)
                nc.vector.tensor_tensor(t1, alpha, c, OP.mult)
                nc.vector.scalar_tensor_tensor(c, t1, neg_h, c, OP.mult, OP.add)
            d_t = smpool.tile([128, SPM], F32, tag="d")
            nc.vector.tensor_scalar(d_t, c, -1.0, 1.0, OP.mult, OP.add)
            st["c"] = c
            st["d_t"] = d_t

        def emit_blend_store(st):
            """xb <- c*xb + (1-c)*tgt in place (bf16), store from ACT queue."""
            xbs, c, d_t, r0 = st["xbs"], st["c"], st["d_t"], st["r0"]
            for s in range(SPM):
                ttmp = ttpool.tile([128, D], BF16, tag="ttmp")
                nc.vector.tensor_scalar(ttmp, tgr_sb, d_t[:, s : s + 1],
                                        None, OP.mult)
                nc.vector.tensor_scalar(xbs[s], xbs[s], c[:, s : s + 1],
                                        None, OP.mult)
                nc.vector.tensor_tensor(xbs[s], xbs[s], ttmp, OP.add)
                nc.scalar.dma_start(
                    out=out_d[r0 + s * SUB : r0 + (s + 1) * SUB, :],
                    in_=xbs[s])

        # Software pipeline (see module docstring).
        xins_q = {0: emit_loads(0), 1: emit_loads(1)}
        xbs_q = {}
        prev = None
        for m in range(nmacro):
            if m + 1 < nmacro:
                xbs_q[m + 1] = emit_casts(xins_q.pop(m + 1))
            if prev is not None:
                emit_blend_store(prev)
            if m == 0:
                xbs_q[0] = emit_casts(xins_q.pop(0))
            if m + 2 < nmacro:
                xins_q[m + 2] = emit_loads(m + 2)
            st = emit_front(m, xbs_q.pop(m))
            emit_iteration(st)
            prev = st
        emit_blend_store(prev)

    if not nc.is_finalized():
        nc.finalize()
    return nc


def _get_program(rows, num_steps, neg_inv, exp_bias, neg_h):
    key = (rows, num_steps, neg_inv, exp_bias, neg_h, DVE_COPY_OF16)
    if key not in _PROGRAM_CACHE:
        _PROGRAM_CACHE[key] = _build_program(rows, num_steps, neg_inv,
                                             exp_bias, neg_h)
    return _PROGRAM_CACHE[key]


def kernel(x, manifold_mu, manifold_U, manifold_S, attractor_mu,
           log_step, sigma, num_steps):
    global LAST_RESULT
    x = np.ascontiguousarray(np.asarray(x, dtype=np.float32))
    mu = np.asarray(manifold_mu, dtype=np.float64)
    U = np.asarray(manifold_U, dtype=np.float64)
    S = np.asarray(manifold_S, dtype=np.float64)
    tgt = np.asarray(attractor_mu, dtype=np.float64)
    ls = float(np.asarray(log_step))
    sg = float(np.asarray(sigma))
    ns = int(np.asarray(num_steps))

    batch, dmodel = x.shape
    assert dmodel == D and batch % N_CORES == 0

    if ns <= 0:
        return x.copy()

    # Host-side parameter folding (O(D*K), trivial). qT/qmu/C use the
    # bf16 W so they are consistent with the device projection, which
    # feeds bf16(x) and bf16(W) into the matmul.
    W = U / (S + 1e-6)[None, :]
    W16 = W.astype(ml_dtypes.bfloat16)
    Wq = W16.astype(np.float64)
    qT = tgt @ Wq
    qmu = mu @ Wq
    wt = qT - qmu
    Cc = float(wt @ wt)
    inv = 1.0 / (float(KSUB) * 2.0 * sg * sg * 1.0)  # TEMPERATURE = 1.0
    step = min(max(math.exp(ls), 1e-3), 1.0)
    h = step / ns

    neg_inv = -inv
    exp_bias = -inv * Cc
    neg_h = -h

    rows = batch // N_CORES
    nc = _get_program(rows, ns, neg_inv, exp_bias, neg_h)

    abr = np.zeros((128, 2), ml_dtypes.bfloat16)
    abr[0:KSUB, 0] = 1.0
    abr[KSUB:128, 1] = (2.0 * wt).astype(ml_dtypes.bfloat16)
    tgr = np.ascontiguousarray(
        np.broadcast_to(tgt.astype(ml_dtypes.bfloat16)[None, :], (128, D)))
    common = {
        "w": np.ascontiguousarray(W16),
        "tgr": tgr,
        "nqt": np.ascontiguousarray((-qT).astype(np.float32)[:, None]),
        "abr": abr,
    }
    in_maps = [
        {"x": x[i * rows : (i + 1) * rows], **common} for i in range(N_CORES)
    ]

    trace = bool(int(os.environ.get("GOF_TRACE", "0")))
    res = run_bass_kernel_spmd(nc, in_maps, list(range(N_CORES)), trace=trace)
    LAST_RESULT = res
    out = np.concatenate([res.results[i]["out"] for i in range(N_CORES)],
                         axis=0)
    return out.astype(np.float32)
